# revision 1
# baseline (speedup 1.0000x reference)
"""BondFastAttention Trainium2 kernel (self-contained).

Shapes (hardcoded from the problem spec):
  edge_attr [65536, 512] fp32, B=64 graphs x L=1024 bonds, HID=512, 8 heads x D=64.
  8 NeuronCores, data-parallel over graphs: G=8 graphs per core.

Device layout: "transposed domain" — features on partitions, tokens on free dim.
Host pre-transposes X and the weights so every matmul contracts on partitions.
f32r (full-rate fp32) for all matmuls. Softmax-over-partitions via small
selector matmuls (seg-sum + expand).
"""
import numpy as np

HID = 512
HEADS = 8
D = 64
B = 64
L = 1024
SCALE = D ** -0.5
EPS = 1e-5
NCORES = 8
G = B // NCORES          # graphs per core
NCH = HID // 128         # 4 feature chunks (2 heads each)
NT = L // 128            # 8 token chunks


def _build(apply_bo: bool, apply_affine: bool):
    import concourse.bass as bass
    from concourse import bacc
    import concourse.mybir as mybir
    from concourse.tile import TileContext

    F32 = mybir.dt.float32
    F32R = mybir.dt.float32r
    BF16 = mybir.dt.bfloat16
    AT = mybir.ActivationFunctionType
    OP = mybir.AluOpType

    nc = bacc.Bacc()

    # Prefer the table set containing BOTH Ln and Exp so every activation in
    # this kernel (Exp/Ln/Copy/Relu) shares one set -> a single table load.
    import concourse.bacc as _bacc_mod
    _orig_gat = _bacc_mod.get_activation_tables

    def _gat(arch):
        # Keep dict order (set id = index into act_info.json) but strip the
        # funcs we use from every other set, so the table-load pass assigns
        # all of them to natural_log_exp_and_others -> one physical load.
        t = _orig_gat(arch)
        ours = {mybir.ActivationFunctionType.Exp, mybir.ActivationFunctionType.Ln,
                mybir.ActivationFunctionType.Copy, mybir.ActivationFunctionType.Relu,
                mybir.ActivationFunctionType.Identity}
        out = {}
        for k, funcs in t.items():
            if k == "natural_log_exp_and_others":
                out[k] = funcs
            else:
                out[k] = {f for f in funcs if f not in ours}
        return out

    xt = nc.dram_tensor("xt", [HID, G * L], BF16, kind="ExternalInput")
    wqst = nc.dram_tensor("wqst", [HID, HID], BF16, kind="ExternalInput")
    wkt = nc.dram_tensor("wkt", [HID, HID], BF16, kind="ExternalInput")
    wvt = nc.dram_tensor("wvt", [HID, HID], BF16, kind="ExternalInput")
    wot = nc.dram_tensor("wot", [HID, HID], BF16, kind="ExternalInput")
    wrbd = nc.dram_tensor("wrbd", [128, 128], BF16, kind="ExternalInput")
    drwa = nc.dram_tensor("drwa", [128, 128], BF16, kind="ExternalInput")
    rwa = nc.dram_tensor("rwa", [128, 1], F32, kind="ExternalInput")
    wbs = nc.dram_tensor("wbs", [128, 1], F32, kind="ExternalInput")
    segs = nc.dram_tensor("segs", [128, 8 * NCH], BF16, kind="ExternalInput")
    sels = nc.dram_tensor("sels", [8, HID], F32, kind="ExternalInput")
    if apply_bo:
        bod = nc.dram_tensor("bo", [1, HID], F32, kind="ExternalInput")
        onesd = nc.dram_tensor("ones1", [1, 128], F32, kind="ExternalInput")
    if apply_affine:
        lngd = nc.dram_tensor("ln_g", [128, HID], F32, kind="ExternalInput")
        lnbd = nc.dram_tensor("ln_b", [128, HID], F32, kind="ExternalInput")
    outd = nc.dram_tensor("out", [G * L, HID], F32, kind="ExternalOutput")

    with TileContext(nc) as tc:
        with tc.tile_pool(name="consts", bufs=1) as cp, \
             tc.tile_pool(name="big", bufs=1) as bp, \
             tc.tile_pool(name="small", bufs=2) as sp, \
             tc.tile_pool(name="pbig", bufs=4, space="PSUM") as ppool, \
             tc.tile_pool(name="pseg", bufs=2, space="PSUM") as spool:

            # ---- constants to SBUF ----

            w_sb = {}
            for name, t in (("q", wqst), ("k", wkt), ("v", wvt), ("o", wot)):
                w_sb[name] = [cp.tile([128, HID], BF16, name=f"w{name}{i}", tag=f"w{name}{i}")
                              for i in range(NCH)]
            for i in range(NCH):
                nc.sync.dma_start(out=w_sb["q"][i], in_=wqst.ap()[128 * i:128 * (i + 1), :])
            wrbd_sb = cp.tile([128, 128], BF16)
            nc.sync.dma_start(out=wrbd_sb, in_=wrbd.ap())
            drwa_sb = cp.tile([128, 128], BF16)
            nc.sync.dma_start(out=drwa_sb, in_=drwa.ap())
            rwa_sb = cp.tile([128, 1], F32)
            nc.sync.dma_start(out=rwa_sb, in_=rwa.ap())
            wbs_sb = cp.tile([128, 1], F32)
            nc.sync.dma_start(out=wbs_sb, in_=wbs.ap())
            segs_sb = cp.tile([128, 8 * NCH], BF16)
            nc.sync.dma_start(out=segs_sb, in_=segs.ap())
            sels_sb = cp.tile([8, HID], F32R)
            nc.sync.dma_start(out=sels_sb, in_=sels.ap().bitcast(F32R))
            eps_sb = cp.tile([128, 1], F32)
            nc.vector.memset(eps_sb, EPS)
            for name, t in (("k", wkt), ("v", wvt), ("o", wot)):
                for i in range(NCH):
                    nc.sync.dma_start(out=w_sb[name][i], in_=t.ap()[128 * i:128 * (i + 1), :])
            if apply_bo:
                ones1_sb = cp.tile([1, 128], F32R)
                nc.sync.dma_start(out=ones1_sb, in_=onesd.ap().bitcast(F32R))
                bo_sb = cp.tile([1, HID], F32R)
                nc.sync.dma_start(out=bo_sb, in_=bod.ap().bitcast(F32R))
            if apply_affine:
                lng_sb = cp.tile([128, HID], F32)
                nc.sync.dma_start(out=lng_sb, in_=lngd.ap())
                lnb_sb = cp.tile([128, HID], F32)
                nc.sync.dma_start(out=lnb_sb, in_=lnbd.ap())

            for g in range(G):
                # ---- load X.T for this graph: [128, (i, l)] ----
                xt_all = bp.tile([128, NCH * L], BF16, name=f"xt{g}", tag="xt", bufs=3)
                xt_src = bass.AP(
                    tensor=xt.ap().tensor, offset=g * L,
                    ap=[[G * L, 128], [128 * G * L, NCH], [1, L]])
                nc.sync.dma_start(
                    out=xt_all.rearrange("p (i l) -> p i l", i=NCH),
                    in_=xt_src)

                def proj_half(wname, j, n0, pp):
                    """half chunk (j, n0) of X @ W.T transposed -> psum [128, 512]"""
                    for i in range(NCH):
                        nc.tensor.matmul(
                            pp,
                            w_sb[wname][i][:, 128 * j:128 * (j + 1)],
                            xt_all[:, i * L + n0: i * L + n0 + 512],
                            start=(i == 0), stop=(i == NCH - 1))

                def softmax_sums(e_t, tag):
                    """seg-sum over head partitions + reciprocal -> (r_lo, r_hi) f32r"""
                    s_lo = spool.tile([8, 512], F32, name=f"slo_{tag}{g}", tag="so", padded_shape=[128, 512])
                    s_hi = spool.tile([8, 512], F32, name=f"shi_{tag}{g}", tag="so", padded_shape=[128, 512])
                    for n0, s_ps in ((0, s_lo), (512, s_hi)):
                        for j in range(NCH):
                            nc.tensor.matmul(
                                s_ps, segs_sb[:, 8 * j:8 * (j + 1)],
                                e_t[:, j * L + n0: j * L + n0 + 512],
                                start=(j == 0), stop=(j == NCH - 1))
                    r_lo = sp.tile([8, 512], F32R, name=f"rlo_{tag}{g}", tag="rlo")
                    r_hi = sp.tile([8, 512], F32R, name=f"rhi_{tag}{g}", tag="rhi")
                    rt = sp.tile([8, 512], F32, name=f"rt_{tag}{g}", tag="rt")
                    nc.vector.reciprocal_approx_fast(out=rt, in_=s_lo)
                    nc.vector.tensor_copy(out=r_lo, in_=rt)
                    rt2 = sp.tile([8, 512], F32, name=f"rt2_{tag}{g}", tag="rt")
                    nc.vector.reciprocal_approx_fast(out=rt2, in_=s_hi)
                    nc.vector.tensor_copy(out=r_hi, in_=rt2)
                    return r_lo, r_hi

                # ================= Q (scaled) + stage A =================
                qs_all = bp.tile([128, NCH * L], BF16, name=f"qs{g}", tag="qs", bufs=3)
                e_all = bp.tile([128, NCH * L], BF16, name=f"ea{g}", tag="e", bufs=3)
                for j in range(NCH):
                    for n0 in (0, 512):
                        pp = ppool.tile([128, 512], F32, name=f"ppq{g}{j}{n0}", tag="pp")
                        proj_half("q", j, n0, pp)
                        nc.scalar.activation(out=e_all[:, j * L + n0: j * L + n0 + 512],
                                             in_=pp, func=AT.Exp)
                        nc.scalar.copy(
                            out=qs_all[:, j * L + n0: j * L + n0 + 512], in_=pp)

                r_lo, r_hi = softmax_sums(e_all, "a")

                # expand + weighted sums -> gq[d] = rwa * sum_l e*rbc*qs
                gq_all = sp.tile([128, NCH], F32, name=f"gq{g}", tag="gq")
                scrs = []
                for j in range(NCH):
                    scr = sp.tile([128, L], BF16, name=f"scra{g}{j}", tag="scr", bufs=4)
                    # m = e * qs   (gpsimd; independent of softmax sums)
                    nc.gpsimd.tensor_mul(
                        out=scr, in0=e_all[:, j * L:(j + 1) * L],
                        in1=qs_all[:, j * L:(j + 1) * L])
                    scrs.append(scr)
                for j in range(NCH):
                    rbc = ppool.tile([128, L], F32, name=f"rbca{g}{j}", tag="rbc", bufs=1)
                    nc.tensor.matmul(rbc[:, 0:512], sels_sb[:, 128 * j:128 * (j + 1)], r_lo)
                    nc.tensor.matmul(rbc[:, 512:L], sels_sb[:, 128 * j:128 * (j + 1)], r_hi)
                    nc.vector.scalar_tensor_tensor(
                        out=scrs[j], in0=scrs[j], scalar=rwa_sb, in1=rbc,
                        op0=OP.mult, op1=OP.mult, accum_out=gq_all[:, j:j + 1])

                # ================= K + stage B =================
                kt_all = bp.tile([128, NCH * L], BF16, name=f"kt{g}", tag="kt", bufs=2)
                eb_all = bp.tile([128, NCH * L], BF16, name=f"eb{g}", tag="e", bufs=3)
                gqwb = sp.tile([128, NCH], F32, name=f"gqwb{g}", tag="gqwb")
                for j in range(NCH):
                    nc.vector.tensor_scalar_mul(out=gqwb[:, j:j + 1],
                                                in0=gq_all[:, j:j + 1], scalar1=wbs_sb)
                for j in range(NCH):
                    for n0 in (0, 512):
                        pp = ppool.tile([128, 512], F32, name=f"ppk{g}{j}{n0}", tag="pp")
                        proj_half("k", j, n0, pp)
                        nc.scalar.copy(out=kt_all[:, j * L + n0: j * L + n0 + 512], in_=pp)
                    nc.scalar.activation(
                        out=eb_all[:, j * L:(j + 1) * L], in_=kt_all[:, j * L:(j + 1) * L],
                        func=AT.Exp, scale=gqwb[:, j:j + 1])

                rb_lo, rb_hi = softmax_sums(eb_all, "b")

                acc_all = sp.tile([128, NCH], F32, name=f"acc{g}", tag="acc")
                scrs_b = []
                for j in range(NCH):
                    scr = sp.tile([128, L], BF16, name=f"scrb{g}{j}", tag="scr", bufs=4)
                    nc.gpsimd.tensor_mul(
                        out=scr, in0=eb_all[:, j * L:(j + 1) * L],
                        in1=kt_all[:, j * L:(j + 1) * L])
                    scrs_b.append(scr)
                for j in range(NCH):
                    rbc = ppool.tile([128, L], F32, name=f"rbcb{g}{j}", tag="rbc", bufs=1)
                    nc.tensor.matmul(rbc[:, 0:512], sels_sb[:, 128 * j:128 * (j + 1)], rb_lo)
                    nc.tensor.matmul(rbc[:, 512:L], sels_sb[:, 128 * j:128 * (j + 1)], rb_hi)
                    nc.vector.scalar_tensor_tensor(
                        out=scrs_b[j], in0=scrs_b[j], scalar=1.0, in1=rbc,
                        op0=OP.mult, op1=OP.mult, accum_out=acc_all[:, j:j + 1])
                gk_all = sp.tile([128, NCH], F32, name=f"gk{g}", tag="gk")
                for j in range(NCH):
                    nc.vector.tensor_mul(out=gk_all[:, j:j + 1],
                                         in0=acc_all[:, j:j + 1], in1=gq_all[:, j:j + 1])

                # ================= V + stage C =================
                vt_all = bp.tile([128, NCH * L], BF16, name=f"vt{g}", tag="vt")
                kv_all = bp.tile([128, NCH * L], BF16, name=f"kv{g}", tag="kv")
                att_all = bp.tile([128, NCH * L], BF16, name=f"att{g}", tag="att", bufs=2)
                for j in range(NCH):
                    for n0 in (0, 512):
                        pp = ppool.tile([128, 512], F32, name=f"ppv{g}{j}{n0}", tag="pp")
                        proj_half("v", j, n0, pp)
                        nc.scalar.copy(out=vt_all[:, j * L + n0: j * L + n0 + 512], in_=pp)
                    kvch = kv_all[:, j * L:(j + 1) * L]
                    # kv = gk * v  (gpsimd, sbuf)
                    nc.gpsimd.tensor_scalar_mul(
                        out=kvch, in0=vt_all[:, j * L:(j + 1) * L],
                        scalar1=gk_all[:, j:j + 1])
                    kvout = ppool.tile([128, L], F32, name=f"kvo{g}{j}", tag="rbc", bufs=1)
                    for n0 in (0, 512):
                        nc.tensor.matmul(kvout[:, n0:n0 + 512], wrbd_sb,
                                         kvch[:, n0:n0 + 512],
                                         start=True, stop=False)
                        nc.tensor.matmul(kvout[:, n0:n0 + 512], drwa_sb,
                                         qs_all[:, j * L + n0: j * L + n0 + 512],
                                         start=False, stop=True)
                    nc.scalar.activation(out=att_all[:, j * L:(j + 1) * L], in_=kvout,
                                         func=AT.Relu)

                # ================= Wo + LayerNorm =================
                mv_all = sp.tile([128, 2 * NT], F32, name=f"mv{g}", tag="mv")
                osb_all = bp.tile([128, NT * HID], F32, name=f"osba{g}", tag="osb", bufs=1)
                for t in range(NT):
                    o_ps = ppool.tile([128, HID], F32, name=f"ops{g}{t}", tag="pp")
                    last = NCH - 1
                    for j in range(NCH):
                        nc.tensor.matmul(
                            o_ps, att_all[:, j * L + 128 * t: j * L + 128 * (t + 1)],
                            w_sb["o"][j], start=(j == 0),
                            stop=(j == last and not apply_bo))
                    if apply_bo:
                        nc.tensor.matmul(o_ps, ones1_sb, bo_sb, start=False, stop=True)
                    och = osb_all[:, t * HID:(t + 1) * HID]
                    nc.scalar.copy(out=och, in_=o_ps)
                    stats = sp.tile([128, 6], F32, name=f"st{g}{t}", tag="st")
                    nc.vector.bn_stats(out=stats, in_=och)
                    nc.vector.bn_aggr(out=mv_all[:, 2 * t:2 * t + 2], in_=stats)
                # per-chunk rstd = exp(-0.5 * ln(var + eps))  (one ACT table set)
                vf = sp.tile([128, NT], F32, name=f"vf{g}", tag="vf")
                lnv = sp.tile([128, NT], F32, name=f"lnv{g}", tag="lnv")
                rstd_all = sp.tile([128, NT], F32, name=f"rstd{g}", tag="rstd")
                for t in range(NT):
                    nc.vector.tensor_scalar_add(out=vf[:, t:t + 1],
                                                in0=mv_all[:, 2 * t + 1:2 * t + 2],
                                                scalar1=EPS)
                    nc.scalar.activation(out=lnv[:, t:t + 1], in_=vf[:, t:t + 1], func=AT.Ln)
                    nc.scalar.activation(out=rstd_all[:, t:t + 1], in_=lnv[:, t:t + 1],
                                         func=AT.Exp, scale=-0.5)
                    och = osb_all[:, t * HID:(t + 1) * HID]
                    nc.vector.tensor_scalar(
                        out=och, in0=och, scalar1=mv_all[:, 2 * t:2 * t + 1],
                        scalar2=rstd_all[:, t:t + 1], op0=OP.subtract, op1=OP.mult)
                    if apply_affine:
                        nc.vector.tensor_mul(out=och, in0=och, in1=lng_sb)
                        nc.vector.tensor_add(out=och, in0=och, in1=lnb_sb)
                    nc.sync.dma_start(
                        out=outd.ap()[g * L + 128 * t: g * L + 128 * (t + 1), :], in_=och)

    _bacc_mod.get_activation_tables = _gat
    try:
        nc.compile()
    finally:
        _bacc_mod.get_activation_tables = _orig_gat
    return nc


_NC_CACHE = {}


def _get_nc(apply_bo, apply_affine):
    key = (apply_bo, apply_affine)
    if key not in _NC_CACHE:
        _NC_CACHE[key] = _build(apply_bo, apply_affine)
    return _NC_CACHE[key]


def _host_consts(Wq, Wk, Wv, Wr, w_alpha, w_beta, Wo, bo, ln_g, ln_b):
    wa_vec = (np.tile(w_alpha, HEADS) * SCALE).astype(np.float32)        # [512]
    wqst = np.ascontiguousarray((Wq * wa_vec[:, None]).T)                 # [h, o]
    wkt = np.ascontiguousarray(Wk.T)
    wvt = np.ascontiguousarray(Wv.T)
    wot = np.ascontiguousarray(Wo.T)
    wrt = Wr.T.astype(np.float32)
    wrbd = np.zeros((128, 128), np.float32)
    wrbd[:64, :64] = wrt; wrbd[64:, 64:] = wrt
    rwa_half = (1.0 / (w_alpha * SCALE)).astype(np.float32)               # [64]
    rwa_col = np.tile(rwa_half, 2).reshape(128, 1)
    drwa = np.diag(np.tile(rwa_half, 2)).astype(np.float32)
    wbs_col = np.tile(w_beta * SCALE, 2).reshape(128, 1).astype(np.float32)
    segs = np.zeros((128, 8 * NCH), np.float32)
    sels = np.zeros((8, HID), np.float32)
    for j in range(NCH):
        for p in range(128):
            segs[p, 8 * j + 2 * j + p // 64] = 1.0
        for m in range(HID):
            if m // 128 == j:
                sels[2 * j + (m % 128) // 64, m] = 1.0
    import ml_dtypes
    bf = ml_dtypes.bfloat16
    common = {"wqst": wqst.astype(bf), "wkt": wkt.astype(bf),
              "wvt": wvt.astype(bf), "wot": wot.astype(bf),
              "wrbd": wrbd.astype(bf), "drwa": drwa.astype(bf), "rwa": rwa_col, "wbs": wbs_col,
              "segs": segs.astype(bf), "sels": sels}
    apply_bo = not np.allclose(bo, 0.0)
    apply_affine = not (np.allclose(ln_g, 1.0) and np.allclose(ln_b, 0.0))
    if apply_bo:
        common["bo"] = bo.reshape(1, HID).astype(np.float32)
        common["ones1"] = np.ones((1, 128), np.float32)
    if apply_affine:
        common["ln_g"] = np.tile(ln_g, (128, 1)).astype(np.float32)
        common["ln_b"] = np.tile(ln_b, (128, 1)).astype(np.float32)
    return common, apply_bo, apply_affine


def kernel(edge_attr, batch_scopes, Wq, Wk, Wv, Wr, w_alpha, w_beta, Wo, bo,
           ln_g, ln_b):
    from concourse import bass_utils

    edge_attr = np.asarray(edge_attr, dtype=np.float32)
    scopes = np.asarray(batch_scopes)
    Wq = np.asarray(Wq, np.float32); Wk = np.asarray(Wk, np.float32)
    Wv = np.asarray(Wv, np.float32); Wr = np.asarray(Wr, np.float32)
    Wo = np.asarray(Wo, np.float32)
    w_alpha = np.asarray(w_alpha, np.float32); w_beta = np.asarray(w_beta, np.float32)
    bo = np.asarray(bo, np.float32)
    ln_g = np.asarray(ln_g, np.float32); ln_b = np.asarray(ln_b, np.float32)

    assert np.all(scopes[:, 1] == L), "equal-length contiguous scopes expected"
    starts = scopes[:, 0].astype(np.int64)

    common, apply_bo, apply_affine = _host_consts(
        Wq, Wk, Wv, Wr, w_alpha, w_beta, Wo, bo, ln_g, ln_b)
    nc = _get_nc(apply_bo, apply_affine)

    in_maps = []
    for c in range(NCORES):
        rows = np.concatenate([
            np.arange(starts[c * G + g], starts[c * G + g] + L)
            for g in range(G)])
        import ml_dtypes
        xslab = edge_attr[rows]                       # [G*L, 512]
        xt = np.ascontiguousarray(xslab.T).astype(ml_dtypes.bfloat16)
        in_maps.append({"xt": xt, **common})

    res = bass_utils.run_bass_kernel_spmd(nc, in_maps, core_ids=list(range(NCORES)))
    out = np.concatenate([r["out"] for r in res.results], axis=0)
    return out.astype(np.float32)

